# revision 44
# baseline (speedup 1.0000x reference)
"""Single-step LSTM cell (B=131072, E=H=128) on 8 Trainium2 NeuronCores.

Strategy: pure data-parallel over the batch. Each core handles 16384 rows.
Host-side we pre-transpose each shard (x^T, h^T, c^T: [128, Bc] bf16) so the
contraction dim (E/H) lands on SBUF partitions — no on-chip transposes.

Per group the four gate pre-activations are computed as per-gate PSUM
tiles [128, w] (2-slot ring over the 8 PSUM banks) with bf16 FD=512
matmuls (W@x, U@h accumulate; matmul output must fit one PSUM bank).
The gate bias enters through the ACT engine's per-partition bias
operand on the activation instruction itself — no bias matmuls at all.
Each gate gets one ACTIVATE (Sigmoid for f/i/o, Tanh directly for c~),
and tanh(c) is one more ACTIVATE from SBUF, software-pipelined one
group late so the strict ACT FIFO never head-of-line blocks on the DVE
chain. Elementwise (f*c_prev + i*c~, o*tanh(c)) runs on DVE fully in
bf16 (2x mode). The c path moves as bf16 both ways, halving its HBM
traffic vs fp32.

Steady state is ACT-bound at the engine's hardware floor (1 elem/lane/
cycle @ 1.2 GHz, 5 activation values per batch element): ~81us busy per
core, with PE (~60us incl. LDWEIGHTS), DVE (~42us) and DMA (20 MiB,
~62us) hidden underneath. Group widths taper 512/1536 at both ends to
shorten the fill and drain phases; ~28 dependency-free junk matmuls at
startup hold the PE busy so its HAM clock-gate reaches 2.4 GHz as the
first real matmuls arrive; the sigmoid/tanh ACT table set is preloaded
via a const-AP activation, and the first x/h transfers dispatch on the
ACT engine's HWDGE queue in parallel with the Sync queue.
"""

import numpy as np

B, E, H = 131072, 128, 128
NCORES = 8
BC = B // NCORES        # 16384 batch rows per core
GROUP = 2048            # batch cols per pipeline group
HALF = GROUP // 2       # matmul moving-operand FD (bf16 max 1024)
NG = BC // GROUP

_CACHE = {}

# gate order in the weight/bias concatenation AND in per-group compute:
# f first (unblocks m1 = f*c_prev early), then c~, then i, then o.
GF, GC, GI, GO = 0, 1, 2, 3


def _build_nc():
    import concourse.bacc as bacc
    import concourse.mybir as mybir
    import concourse.tile as tile

    f32 = mybir.dt.float32
    bf = mybir.dt.bfloat16
    AF = mybir.ActivationFunctionType

    nc = bacc.Bacc("TRN2", target_bir_lowering=False, debug=False,
                   num_devices=NCORES)

    xT = nc.dram_tensor("xT", [E, BC], bf, kind="ExternalInput").ap()
    hT = nc.dram_tensor("hT", [H, BC], bf, kind="ExternalInput").ap()
    cT = nc.dram_tensor("cT", [H, BC], bf, kind="ExternalInput").ap()
    W = nc.dram_tensor("W", [E, 4 * H], bf, kind="ExternalInput").ap()
    U = nc.dram_tensor("U", [H, 4 * H], bf, kind="ExternalInput").ap()
    bias = nc.dram_tensor("b", [H, 4], f32, kind="ExternalInput").ap()
    hT_out = nc.dram_tensor("hT_out", [H, BC], bf, kind="ExternalOutput").ap()
    cT_out = nc.dram_tensor("cT_out", [H, BC], bf, kind="ExternalOutput").ap()

    with tile.TileContext(nc) as tc:
        with tc.tile_pool(name="cst", bufs=1) as cst, \
             tc.tile_pool(name="xin", bufs=3) as xin, \
             tc.tile_pool(name="hin", bufs=3) as hin, \
             tc.tile_pool(name="cin", bufs=3) as cin, \
             tc.tile_pool(name="hout", bufs=2) as hout, \
             tc.tile_pool(name="cout", bufs=2) as cout, \
             tc.tile_pool(name="work", bufs=2) as work, \
             tc.tile_pool(name="ps", bufs=2, space="PSUM") as ps:

            W_sb = cst.tile([E, 4 * H], bf)
            U_sb = cst.tile([H, 4 * H], bf)
            b_sb = cst.tile([H, 4], f32)

            # first data + constants moving before anything else
            x0_sb = xin.tile([E, GROUP], bf, name="x0_sb", tag="x_sb")
            h0_sb = hin.tile([H, GROUP], bf, name="h0_sb", tag="h_sb")
            c0_sb = cin.tile([H, GROUP], bf, name="c0_sb", tag="c_sb")
            # dispatch the critical first transfers on the ACT engine's
            # HWDGE queue, in parallel with the Sync queue (dispatch is
            # serial ~600ns per DMA within one queue)
            nc.scalar.dma_start(out=x0_sb[:, 0:512], in_=xT[:, 0:512])
            nc.scalar.dma_start(out=h0_sb[:, 0:512], in_=hT[:, 0:512])
            nc.sync.dma_start(out=W_sb[:], in_=W)
            nc.sync.dma_start(out=U_sb[:], in_=U)
            nc.sync.dma_start(out=b_sb[:], in_=bias)
            nc.sync.dma_start(out=c0_sb[:, 0:512], in_=cT[:, 0:512])

            # preload the sigmoid/tanh ACT table set while the first
            # group's DMA is in flight (the table swap costs ~2.7us).
            # Read a pre-initialized const AP: no DMA dependency, so it
            # fires the moment the ACT engine preamble finishes.
            dmy = cst.tile([E, 1], bf, name="dmy")
            czero = nc.const_aps.aps[(mybir.dt.float32, 0.0)]
            nc.scalar.activation(dmy[:], czero, AF.Sigmoid)

            # warm the PE clock (HAM un-throttles after ~5us of sustained
            # activity): feed it junk matmuls with no data dependency at
            # all (uninitialized SBUF is fine — nothing reads the result),
            # so they start right after the PE preamble and finish about
            # when the first real data lands
            wsrc = cst.tile([E, 512], bf, name="wsrc")
            nc.vector.memset(wsrc[:], 1.0)
            warm = ps.tile([H, GROUP], f32, name="warm", tag="ps")
            for _ in range(16):
                nc.tensor.matmul(warm[:, 0:128], wsrc[:, 0:H],
                                 wsrc[:, 0:128], start=True, stop=True)

            # variable-width groups: taper at both ends (earlier first
            # activation; shorter drain tail). Middle groups full width.
            widths = [512, 1536] + [GROUP] * (NG - 2) + [1536, 512]
            offs = [sum(widths[:i]) for i in range(len(widths))]

            # one tanh per group: a bigger batched tanh blocks the ACT
            # FIFO for too long and stalls the gate pipeline (measured)
            pair_plan = [(ch,) for ch in range(len(widths))]
            pair_start = {p[0]: p for p in pair_plan}
            pair_last = {p[-1] for p in pair_plan}

            def emit_tail(pair):
                """tanh(c) + h = o*tanh(c) + h DMA-out for finished groups.

                Deferred one pair so the ACT-FIFO tanh never waits on the
                DVE chain of its own group (head-of-line blocking).
                """
                members, co_sb, total = pair
                tc_sb = work.tile([H, total], bf, tag="tc")
                nc.scalar.activation(tc_sb[:], co_sb[:, 0:total], AF.Tanh)
                lo = 0
                for p_off, p_w, p_o in members:
                    ho_sb = hout.tile([H, p_w], bf, tag="ho")
                    nc.vector.tensor_mul(out=ho_sb[:], in0=p_o[:, 0:p_w],
                                         in1=tc_sb[:, lo:lo + p_w])
                    nparts = 2 if p_w <= 1024 else 1
                    PC = p_w // nparts
                    for hf in range(nparts):
                        o2, o3 = hf * PC, p_off + hf * PC
                        nc.sync.dma_start(out=hT_out[:, o3:o3 + PC],
                                          in_=ho_sb[:, o2:o2 + PC])
                    lo += p_w

            prev = None
            cur = None
            for ch, (off, w) in enumerate(zip(offs, widths)):
                if ch == 0:
                    x_sb, h_sb, c_sb = x0_sb, h0_sb, c0_sb
                else:
                    x_sb = xin.tile([E, GROUP], bf, tag="x_sb")
                    h_sb = hin.tile([H, GROUP], bf, tag="h_sb")
                    c_sb = cin.tile([H, GROUP], bf, tag="c_sb")
                    for o2 in range(0, w, HALF):
                        pw = min(HALF, w - o2)
                        o3 = off + o2
                        nc.sync.dma_start(out=x_sb[:, o2:o2 + pw],
                                          in_=xT[:, o3:o3 + pw])
                        nc.sync.dma_start(out=h_sb[:, o2:o2 + pw],
                                          in_=hT[:, o3:o3 + pw])
                    nc.sync.dma_start(out=c_sb[:, 0:w], in_=cT[:, off:off + w])

                sig = {}
                for g in (GF, GC, GI, GO):
                    Wg = W_sb[:, g * H:(g + 1) * H]
                    Ug = U_sb[:, g * H:(g + 1) * H]
                    ps_g = ps.tile([H, GROUP], f32, tag="ps")
                    for q in range(w // 512):
                        qo = q * 512
                        nc.tensor.matmul(ps_g[:, qo:qo + 512], Wg,
                                         x_sb[:, qo:qo + 512],
                                         start=True, stop=False)
                    for q in range(w // 512):
                        qo = q * 512
                        nc.tensor.matmul(ps_g[:, qo:qo + 512], Ug,
                                         h_sb[:, qo:qo + 512],
                                         start=False, stop=True)
                    s_g = work.tile([H, GROUP], bf, tag=f"sig{g}",
                                    bufs=3 if g == GO else 2)
                    fn = AF.Tanh if g == GC else AF.Sigmoid
                    nc.scalar.activation(s_g[:, 0:w], ps_g[:, 0:w], fn,
                                         bias=b_sb[:, g:g + 1])
                    sig[g] = s_g
                    if g == GC and prev is not None and ch >= NG:
                        # final groups: slot the previous tanh between
                        # the gate activations so it doesn't serialize
                        # into the drain tail
                        emit_tail(prev)
                        prev = None

                if ch in pair_start:
                    total = sum(widths[m] for m in pair_start[ch])
                    co_pair = cout.tile([H, total], bf, name="co_pair",
                                        tag="co")
                    cur = ([], co_pair, total, off)
                members, co_sb, total, base = cur
                lo = off - base
                m1 = work.tile([H, GROUP], bf, tag="m1")
                m2 = work.tile([H, GROUP], bf, tag="m2")
                nc.vector.tensor_mul(out=m1[:, 0:w], in0=sig[GF][:, 0:w],
                                     in1=c_sb[:, 0:w])
                nc.vector.tensor_mul(out=m2[:, 0:w], in0=sig[GI][:, 0:w],
                                     in1=sig[GC][:, 0:w])
                nc.vector.tensor_add(out=co_sb[:, lo:lo + w],
                                     in0=m1[:, 0:w], in1=m2[:, 0:w])
                nc.sync.dma_start(out=cT_out[:, off:off + w],
                                  in_=co_sb[:, lo:lo + w])
                members.append((off, w, sig[GO]))
                # previous group's tanh(c)+h goes AFTER this group's
                # gate activations: its input has long been ready, and
                # trailing it gives the PE two extra ACTIVATE-durations
                # of margin to fill the next group's first PSUM tile
                if prev is not None:
                    emit_tail(prev)
                    prev = None
                if ch in pair_last:
                    prev = (members, co_sb, total)
                    cur = None

            emit_tail(prev)

    nc.compile()
    return nc


def kernel(x, hidden_memory_tm1, Wi, Ui, bi, Wf, Uf, bf, Wog, Uog, bog,
           Wc, Uc, bc, _return_timing=False, _trace=False):
    from concourse.bass_utils import run_bass_kernel_spmd

    if "nc" not in _CACHE:
        _CACHE["nc"] = _build_nc()
    nc = _CACHE["nc"]

    import ml_dtypes
    bf16 = ml_dtypes.bfloat16
    x = np.asarray(x, np.float32)
    hm = np.asarray(hidden_memory_tm1, np.float32)
    # gate order f, c~, i, o (c~ uses Tanh directly on the ACT engine)
    W = np.concatenate([Wf, Wc, Wi, Wog], axis=1).astype(bf16)
    U = np.concatenate([Uf, Uc, Ui, Uog], axis=1).astype(bf16)
    b = np.stack([np.asarray(bf), np.asarray(bc),
                  np.asarray(bi), np.asarray(bog)], axis=1).astype(np.float32)
    b = np.ascontiguousarray(b)  # [H, 4], column g = per-partition bias

    in_maps = []
    for c in range(NCORES):
        sl = slice(c * BC, (c + 1) * BC)
        in_maps.append({
            "xT": np.ascontiguousarray(x[sl].astype(bf16).T),
            "hT": np.ascontiguousarray(hm[0, sl].astype(bf16).T),
            "cT": np.ascontiguousarray(hm[1, sl].astype(bf16).T),
            "W": W, "U": U, "b": b,
        })

    res = run_bass_kernel_spmd(nc, in_maps, core_ids=list(range(NCORES)),
                               trace=_trace)

    h = np.concatenate(
        [res.results[c]["hT_out"].T.astype(np.float32) for c in range(NCORES)], 0)
    cc = np.concatenate(
        [res.results[c]["cT_out"].T.astype(np.float32) for c in range(NCORES)], 0)
    out = np.stack([h, cc])
    if _return_timing:
        return out, res
    return out


# revision 45
# speedup vs baseline: 1.0430x; 1.0430x over previous
"""Single-step LSTM cell (B=131072, E=H=128) on 8 Trainium2 NeuronCores.

Strategy: pure data-parallel over the batch. Each core handles 16384 rows.
Host-side we pre-transpose each shard (x^T, h^T, c^T: [128, Bc] bf16) so the
contraction dim (E/H) lands on SBUF partitions — no on-chip transposes.

Per group the four gate pre-activations are computed as per-gate PSUM
tiles [128, w] (2-slot ring over the 8 PSUM banks) with bf16 FD=512
matmuls (W@x, U@h accumulate; matmul output must fit one PSUM bank).
The gate bias enters through the ACT engine's per-partition bias
operand on the activation instruction itself — no bias matmuls at all.
Each gate gets one ACTIVATE (Sigmoid for f/i/o, Tanh directly for c~),
and tanh(c) is one more ACTIVATE from SBUF, software-pipelined one
group late so the strict ACT FIFO never head-of-line blocks on the DVE
chain. Elementwise (f*c_prev + i*c~, o*tanh(c)) runs on DVE fully in
bf16 (2x mode). The c path moves as bf16 both ways, halving its HBM
traffic vs fp32.

Steady state is ACT-bound at the engine's hardware floor (1 elem/lane/
cycle @ 1.2 GHz, 5 activation values per batch element): ~81us busy per
core, with PE (~60us incl. LDWEIGHTS), DVE (~42us) and DMA (20 MiB,
~62us) hidden underneath. Group widths taper 512/1536 at both ends to
shorten the fill and drain phases; ~28 dependency-free junk matmuls at
startup hold the PE busy so its HAM clock-gate reaches 2.4 GHz as the
first real matmuls arrive; the sigmoid/tanh ACT table set is preloaded
via a const-AP activation, and the first x/h transfers dispatch on the
ACT engine's HWDGE queue in parallel with the Sync queue.
"""

import numpy as np

B, E, H = 131072, 128, 128
NCORES = 8
BC = B // NCORES        # 16384 batch rows per core
GROUP = 2048            # batch cols per pipeline group
HALF = GROUP // 2       # matmul moving-operand FD (bf16 max 1024)
NG = BC // GROUP

_CACHE = {}

# gate order in the weight/bias concatenation AND in per-group compute:
# f first (unblocks m1 = f*c_prev early), then c~, then i, then o.
GF, GC, GI, GO = 0, 1, 2, 3


def _build_nc():
    import concourse.bacc as bacc
    import concourse.mybir as mybir
    import concourse.tile as tile

    f32 = mybir.dt.float32
    bf = mybir.dt.bfloat16
    AF = mybir.ActivationFunctionType

    nc = bacc.Bacc("TRN2", target_bir_lowering=False, debug=False,
                   num_devices=NCORES)

    xT = nc.dram_tensor("xT", [E, BC], bf, kind="ExternalInput").ap()
    hT = nc.dram_tensor("hT", [H, BC], bf, kind="ExternalInput").ap()
    cT = nc.dram_tensor("cT", [H, BC], bf, kind="ExternalInput").ap()
    W = nc.dram_tensor("W", [E, 4 * H], bf, kind="ExternalInput").ap()
    U = nc.dram_tensor("U", [H, 4 * H], bf, kind="ExternalInput").ap()
    bias = nc.dram_tensor("b", [H, 4], f32, kind="ExternalInput").ap()
    hT_out = nc.dram_tensor("hT_out", [H, BC], bf, kind="ExternalOutput").ap()
    cT_out = nc.dram_tensor("cT_out", [H, BC], bf, kind="ExternalOutput").ap()

    with tile.TileContext(nc) as tc:
        with tc.tile_pool(name="cst", bufs=1) as cst, \
             tc.tile_pool(name="xin", bufs=3) as xin, \
             tc.tile_pool(name="hin", bufs=3) as hin, \
             tc.tile_pool(name="cin", bufs=3) as cin, \
             tc.tile_pool(name="hout", bufs=2) as hout, \
             tc.tile_pool(name="cout", bufs=2) as cout, \
             tc.tile_pool(name="work", bufs=2) as work, \
             tc.tile_pool(name="ps", bufs=2, space="PSUM") as ps:

            W_sb = cst.tile([E, 4 * H], bf)
            U_sb = cst.tile([H, 4 * H], bf)
            b_sb = cst.tile([H, 4], f32)

            # first data + constants moving before anything else
            x0_sb = xin.tile([E, GROUP], bf, name="x0_sb", tag="x_sb")
            h0_sb = hin.tile([H, GROUP], bf, name="h0_sb", tag="h_sb")
            c0_sb = cin.tile([H, GROUP], bf, name="c0_sb", tag="c_sb")
            # dispatch the critical first transfers on the ACT engine's
            # HWDGE queue, in parallel with the Sync queue (dispatch is
            # serial ~600ns per DMA within one queue)
            nc.scalar.dma_start(out=x0_sb[:, 0:512], in_=xT[:, 0:512])
            nc.scalar.dma_start(out=h0_sb[:, 0:512], in_=hT[:, 0:512])
            nc.sync.dma_start(out=W_sb[:], in_=W)
            nc.sync.dma_start(out=U_sb[:], in_=U)
            nc.sync.dma_start(out=b_sb[:], in_=bias)
            nc.sync.dma_start(out=c0_sb[:, 0:512], in_=cT[:, 0:512])

            # preload the sigmoid/tanh ACT table set while the first
            # group's DMA is in flight (the table swap costs ~2.7us).
            # Read a pre-initialized const AP: no DMA dependency, so it
            # fires the moment the ACT engine preamble finishes.
            dmy = cst.tile([E, 1], bf, name="dmy")
            czero = nc.const_aps.aps[(mybir.dt.float32, 0.0)]
            nc.scalar.activation(dmy[:], czero, AF.Sigmoid)

            # warm the PE clock (HAM un-throttles after ~5us of sustained
            # activity): feed it junk matmuls with no data dependency at
            # all (uninitialized SBUF is fine — nothing reads the result),
            # so they start right after the PE preamble and finish about
            # when the first real data lands
            wsrc = cst.tile([E, 512], bf, name="wsrc")
            nc.vector.memset(wsrc[:], 1.0)
            warm = ps.tile([H, GROUP], f32, name="warm", tag="ps")
            for _ in range(28):
                nc.tensor.matmul(warm[:, 0:128], wsrc[:, 0:H],
                                 wsrc[:, 0:128], start=True, stop=True)

            # variable-width groups: taper at both ends (earlier first
            # activation; shorter drain tail). Middle groups full width.
            widths = [512, 1536] + [GROUP] * (NG - 2) + [1536, 512]
            offs = [sum(widths[:i]) for i in range(len(widths))]

            # one tanh per group: a bigger batched tanh blocks the ACT
            # FIFO for too long and stalls the gate pipeline (measured)
            pair_plan = [(ch,) for ch in range(len(widths))]
            pair_start = {p[0]: p for p in pair_plan}
            pair_last = {p[-1] for p in pair_plan}

            def emit_tail(pair):
                """tanh(c) + h = o*tanh(c) + h DMA-out for finished groups.

                Deferred one pair so the ACT-FIFO tanh never waits on the
                DVE chain of its own group (head-of-line blocking).
                """
                members, co_sb, total = pair
                tc_sb = work.tile([H, total], bf, tag="tc")
                nc.scalar.activation(tc_sb[:], co_sb[:, 0:total], AF.Tanh)
                lo = 0
                for p_off, p_w, p_o in members:
                    ho_sb = hout.tile([H, p_w], bf, tag="ho")
                    nc.vector.tensor_mul(out=ho_sb[:], in0=p_o[:, 0:p_w],
                                         in1=tc_sb[:, lo:lo + p_w])
                    nparts = 2 if p_w <= 1024 else 1
                    PC = p_w // nparts
                    for hf in range(nparts):
                        o2, o3 = hf * PC, p_off + hf * PC
                        nc.sync.dma_start(out=hT_out[:, o3:o3 + PC],
                                          in_=ho_sb[:, o2:o2 + PC])
                    lo += p_w

            prev = None
            cur = None
            for ch, (off, w) in enumerate(zip(offs, widths)):
                if ch == 0:
                    x_sb, h_sb, c_sb = x0_sb, h0_sb, c0_sb
                else:
                    x_sb = xin.tile([E, GROUP], bf, tag="x_sb")
                    h_sb = hin.tile([H, GROUP], bf, tag="h_sb")
                    c_sb = cin.tile([H, GROUP], bf, tag="c_sb")
                    for o2 in range(0, w, HALF):
                        pw = min(HALF, w - o2)
                        o3 = off + o2
                        nc.sync.dma_start(out=x_sb[:, o2:o2 + pw],
                                          in_=xT[:, o3:o3 + pw])
                        nc.sync.dma_start(out=h_sb[:, o2:o2 + pw],
                                          in_=hT[:, o3:o3 + pw])
                    nc.sync.dma_start(out=c_sb[:, 0:w], in_=cT[:, off:off + w])

                sig = {}
                for g in (GF, GC, GI, GO):
                    Wg = W_sb[:, g * H:(g + 1) * H]
                    Ug = U_sb[:, g * H:(g + 1) * H]
                    ps_g = ps.tile([H, GROUP], f32, tag="ps")
                    for q in range(w // 512):
                        qo = q * 512
                        nc.tensor.matmul(ps_g[:, qo:qo + 512], Wg,
                                         x_sb[:, qo:qo + 512],
                                         start=True, stop=False)
                    for q in range(w // 512):
                        qo = q * 512
                        nc.tensor.matmul(ps_g[:, qo:qo + 512], Ug,
                                         h_sb[:, qo:qo + 512],
                                         start=False, stop=True)
                    s_g = work.tile([H, GROUP], bf, tag=f"sig{g}",
                                    bufs=3 if g == GO else 2)
                    fn = AF.Tanh if g == GC else AF.Sigmoid
                    nc.scalar.activation(s_g[:, 0:w], ps_g[:, 0:w], fn,
                                         bias=b_sb[:, g:g + 1])
                    sig[g] = s_g
                    if g == GC and prev is not None and ch >= NG:
                        # final groups: slot the previous tanh between
                        # the gate activations so it doesn't serialize
                        # into the drain tail
                        emit_tail(prev)
                        prev = None

                if ch in pair_start:
                    total = sum(widths[m] for m in pair_start[ch])
                    co_pair = cout.tile([H, total], bf, name="co_pair",
                                        tag="co")
                    cur = ([], co_pair, total, off)
                members, co_sb, total, base = cur
                lo = off - base
                m1 = work.tile([H, GROUP], bf, tag="m1")
                m2 = work.tile([H, GROUP], bf, tag="m2")
                nc.vector.tensor_mul(out=m1[:, 0:w], in0=sig[GF][:, 0:w],
                                     in1=c_sb[:, 0:w])
                nc.vector.tensor_mul(out=m2[:, 0:w], in0=sig[GI][:, 0:w],
                                     in1=sig[GC][:, 0:w])
                nc.vector.tensor_add(out=co_sb[:, lo:lo + w],
                                     in0=m1[:, 0:w], in1=m2[:, 0:w])
                nc.sync.dma_start(out=cT_out[:, off:off + w],
                                  in_=co_sb[:, lo:lo + w])
                members.append((off, w, sig[GO]))
                # previous group's tanh(c)+h goes AFTER this group's
                # gate activations: its input has long been ready, and
                # trailing it gives the PE two extra ACTIVATE-durations
                # of margin to fill the next group's first PSUM tile
                if prev is not None:
                    emit_tail(prev)
                    prev = None
                if ch in pair_last:
                    prev = (members, co_sb, total)
                    cur = None

            emit_tail(prev)

    nc.compile()
    return nc


def kernel(x, hidden_memory_tm1, Wi, Ui, bi, Wf, Uf, bf, Wog, Uog, bog,
           Wc, Uc, bc, _return_timing=False, _trace=False):
    from concourse.bass_utils import run_bass_kernel_spmd

    if "nc" not in _CACHE:
        _CACHE["nc"] = _build_nc()
    nc = _CACHE["nc"]

    import ml_dtypes
    bf16 = ml_dtypes.bfloat16
    x = np.asarray(x, np.float32)
    hm = np.asarray(hidden_memory_tm1, np.float32)
    # gate order f, c~, i, o (c~ uses Tanh directly on the ACT engine)
    W = np.concatenate([Wf, Wc, Wi, Wog], axis=1).astype(bf16)
    U = np.concatenate([Uf, Uc, Ui, Uog], axis=1).astype(bf16)
    b = np.stack([np.asarray(bf), np.asarray(bc),
                  np.asarray(bi), np.asarray(bog)], axis=1).astype(np.float32)
    b = np.ascontiguousarray(b)  # [H, 4], column g = per-partition bias

    in_maps = []
    for c in range(NCORES):
        sl = slice(c * BC, (c + 1) * BC)
        in_maps.append({
            "xT": np.ascontiguousarray(x[sl].astype(bf16).T),
            "hT": np.ascontiguousarray(hm[0, sl].astype(bf16).T),
            "cT": np.ascontiguousarray(hm[1, sl].astype(bf16).T),
            "W": W, "U": U, "b": b,
        })

    res = run_bass_kernel_spmd(nc, in_maps, core_ids=list(range(NCORES)),
                               trace=_trace)

    h = np.concatenate(
        [res.results[c]["hT_out"].T.astype(np.float32) for c in range(NCORES)], 0)
    cc = np.concatenate(
        [res.results[c]["cT_out"].T.astype(np.float32) for c in range(NCORES)], 0)
    out = np.stack([h, cc])
    if _return_timing:
        return out, res
    return out
